# revision 5
# baseline (speedup 1.0000x reference)
"""Trainium2 Bass kernel: autoregressive wavefunction log-prob (N=64, B=2048, H=512).

Sharding: step axis N across 8 cores, round-robin (core c owns global steps
c, c+8, ..., c+56).  Each core computes, for its 8 steps i:
    h1 = relu(X_i @ W1_i + b1_i)        X_i = prefix one-hots (masked into W1)
    h2 = relu(h1 @ W2_i + b2_i)
    d  = h2 @ (W3_i[:,0]-W3_i[:,1]) + (b3_i[0]-b3_i[1])
    logp_i = -softplus(-(sigma_i * d))  sigma = s0 - s1 in {+1,-1}
and returns the [8, B] per-step logp matrix; the host sums over steps/cores.

On-chip layout: activations are kept transposed [H, B] so W1 [2N,H] and
W2 [Hin,Hout] serve directly as matmul lhsT.  The ragged prefix (rows >= 2i
of the padded W1) is zeroed on the host so a single SPMD graph serves all
cores.  All matmuls run in bf16 with fp32 PSUM accumulation.
"""

import numpy as np
import ml_dtypes

import concourse.bass as bass
import concourse.mybir as mybir
import concourse.tile as tile
from concourse.bass_utils import run_bass_kernel_spmd

N, B, H = 64, 2048, 512
NCORES = 8
NSTEP = N // NCORES          # 8 local steps per core
BCH = 512                    # batch chunk (one PSUM bank of fp32)
NB = B // BCH                # 4
NM = H // 128                # 4 h-chunks
K2N = 2 * N                  # 128, layer-1 contraction

BF = mybir.dt.bfloat16
F32 = mybir.dt.float32
NPBF = ml_dtypes.bfloat16

# set by test harness to capture profile/results
TRACE = False
LAST_RESULT = None


def _legalize_waits(nc):
    """This walrus build encodes at most ONE semaphore wait per instruction
    (one NEURON_ISA_TPB_EVENTS slot).  Tile emits multi-wait sync_info; spill
    the extras onto standalone EventSemaphore instructions inserted just
    before, in the same engine's FIFO stream — semantically identical."""
    for fn in nc.m.functions:
        for blk in fn.blocks:
            new = []
            for inst in blk.instructions:
                si = inst.sync_info
                if si is not None and si.on_wait is not None and len(si.on_wait) > 1:
                    waits = list(si.on_wait)
                    for idx, w in enumerate(waits[:-1]):
                        new.append(mybir.InstEventSemaphore(
                            name=f"{inst.name}-spill{idx}",
                            engine=inst.engine,
                            sync_info=mybir.SyncInfo(on_wait=[w], on_update=[]),
                        ))
                    inst.sync_info = mybir.SyncInfo(
                        on_wait=[waits[-1]], on_update=list(si.on_update)
                    )
                new.append(inst)
            blk.instructions = new
    return nc


def build_graph():
    nc = bass.Bass()
    S_d = nc.declare_dram_parameter("S", [K2N, B], BF, False)
    W1_d = nc.declare_dram_parameter("W1", [NSTEP, K2N, H], BF, False)
    W2_d = nc.declare_dram_parameter("W2", [NSTEP, 128, NM * H], BF, False)
    B1_d = nc.declare_dram_parameter("B1", [128, NSTEP * NM], F32, False)
    B2_d = nc.declare_dram_parameter("B2", [128, NSTEP * NM], F32, False)
    W3D_d = nc.declare_dram_parameter("W3D", [128, NSTEP * NM * NSTEP], BF, False)
    B3D_d = nc.declare_dram_parameter("B3D", [NSTEP, 1], F32, False)
    SIG_d = nc.declare_dram_parameter("SIG", [NSTEP, B], F32, False)
    OUT_d = nc.declare_dram_parameter("out", [NSTEP, B], F32, True)

    add = mybir.AluOpType.add
    mult = mybir.AluOpType.mult
    amax = mybir.AluOpType.max

    with tile.TileContext(nc) as tc:
        with (
            tc.tile_pool(name="const", bufs=1) as const,
            tc.tile_pool(name="w1p", bufs=2) as w1p,
            tc.tile_pool(name="w2p", bufs=2) as w2p,
            tc.tile_pool(name="h1p", bufs=18) as h1p,
            tc.tile_pool(name="h2p", bufs=6) as h2p,
            tc.tile_pool(name="tailp", bufs=2) as tailp,
            tc.tile_pool(name="pp", bufs=4, space="PSUM") as pp,
            tc.tile_pool(name="dp", bufs=1, space="PSUM") as dp,
        ):
            S_sb = const.tile([K2N, B], BF)
            nc.sync.dma_start(out=S_sb[:], in_=S_d[:])
            B1_sb = const.tile([128, NSTEP * NM], F32)
            nc.sync.dma_start(out=B1_sb[:], in_=B1_d[:])
            B2_sb = const.tile([128, NSTEP * NM], F32)
            nc.sync.dma_start(out=B2_sb[:], in_=B2_d[:])
            W3D_sb = const.tile([128, NSTEP * NM * NSTEP], BF)
            nc.sync.dma_start(out=W3D_sb[:], in_=W3D_d[:])
            B3D_sb = const.tile([NSTEP, 1], F32)
            nc.sync.dma_start(out=B3D_sb[:], in_=B3D_d[:])
            SIG_sb = const.tile([NSTEP, B], F32)
            nc.sync.dma_start(out=SIG_sb[:], in_=SIG_d[:])

            D = dp.tile([NSTEP, B], F32)  # persistent accumulator, 4 banks

            for j in range(NSTEP):
                w1 = w1p.tile([K2N, H], BF, tag="w1")
                nc.sync.dma_start(out=w1[:], in_=W1_d[j])
                w2 = w2p.tile([128, NM * H], BF, tag="w2")
                nc.sync.dma_start(out=w2[:], in_=W2_d[j])

                # ---- layer 1: h1T[m] = relu(W1[:,m]^T S + b1[m]),  [128, B]
                h1 = {}
                for b in range(NB):
                    bs = slice(b * BCH, (b + 1) * BCH)
                    for m in range(NM):
                        ps = pp.tile([128, BCH], F32, tag="ps")
                        nc.tensor.matmul(
                            ps[:],
                            w1[:, m * 128:(m + 1) * 128],
                            S_sb[:, bs],
                            start=True,
                            stop=True,
                        )
                        t = h1p.tile([128, BCH], BF, tag="h1")
                        bias = B1_sb[:, j * NM + m: j * NM + m + 1]
                        if m % 2 == 0:
                            nc.scalar.activation(
                                t[:], ps[:], mybir.ActivationFunctionType.Relu,
                                bias=bias,
                            )
                        else:
                            nc.vector.tensor_scalar(
                                t[:], ps[:], bias, 0.0, op0=add, op1=amax,
                            )
                        h1[(m, b)] = t

                # ---- layer 2 + layer 3 per batch chunk
                for b in range(NB):
                    bs = slice(b * BCH, (b + 1) * BCH)
                    h2 = {}
                    for m in range(NM):
                        ps = pp.tile([128, BCH], F32, tag="ps")
                        for k in range(NM):
                            nc.tensor.matmul(
                                ps[:],
                                w2[:, k * H + m * 128: k * H + (m + 1) * 128],
                                h1[(k, b)][:],
                                start=(k == 0),
                                stop=(k == NM - 1),
                            )
                        t = h2p.tile([128, BCH], BF, tag="h2")
                        bias = B2_sb[:, j * NM + m: j * NM + m + 1]
                        if m % 2 == 1:
                            nc.scalar.activation(
                                t[:], ps[:], mybir.ActivationFunctionType.Relu,
                                bias=bias,
                            )
                        else:
                            nc.vector.tensor_scalar(
                                t[:], ps[:], bias, 0.0, op0=add, op1=amax,
                            )
                        h2[m] = t

                    # layer 3: D[0:8, bs] += W3D[j,k]^T @ h2[k]
                    for k in range(NM):
                        c0 = (j * NM + k) * NSTEP
                        nc.tensor.matmul(
                            D[0:NSTEP, bs],
                            W3D_sb[:, c0: c0 + NSTEP],
                            h2[k][:],
                            start=(j == 0 and k == 0),
                            stop=(j == NSTEP - 1 and k == NM - 1),
                            skip_group_check=True,
                        )

            # ---- tail: logp = ln(sigmoid(sigma * (d + b3d)))
            tt = tailp.tile([NSTEP, B], F32, tag="tt")
            nc.vector.scalar_tensor_tensor(
                tt[:], D[0:NSTEP, :], B3D_sb[:, 0:1], SIG_sb[:],
                op0=add, op1=mult,
            )
            sg = tailp.tile([NSTEP, B], F32, tag="sg")
            nc.scalar.activation(
                sg[:], tt[:], mybir.ActivationFunctionType.Sigmoid,
            )
            lp = tailp.tile([NSTEP, B], F32, tag="lp")
            nc.scalar.activation(
                lp[:], sg[:], mybir.ActivationFunctionType.Ln,
            )
            nc.sync.dma_start(out=OUT_d[:], in_=lp[:])

    return _legalize_waits(nc)


_NC_CACHE = None


def _get_graph():
    global _NC_CACHE
    if _NC_CACHE is None:
        _NC_CACHE = build_graph()
    return _NC_CACHE


def _prep_inputs(samples, W1, b1, W2, b2, W3, b3):
    samples = np.asarray(samples, np.float32)
    W1 = np.asarray(W1, np.float32)
    b1 = np.asarray(b1, np.float32)
    W2 = np.asarray(W2, np.float32)
    b2 = np.asarray(b2, np.float32)
    W3 = np.asarray(W3, np.float32)
    b3 = np.asarray(b3, np.float32)

    # S[2j+s, b] = samples[j, b, s]
    S = samples.transpose(0, 2, 1).reshape(K2N, B).astype(NPBF)
    # mask padded rows: row k of W1[i] is dead unless k < 2i
    row = np.arange(K2N)[None, :, None]
    step = np.arange(N)[:, None, None]
    W1m = np.where(row < 2 * step, W1, 0.0).astype(NPBF)
    w3d = (W3[:, :, 0] - W3[:, :, 1]).astype(np.float32)      # (N, H)
    b3d = (b3[:, 0] - b3[:, 1]).astype(np.float32)            # (N,)
    sig = (samples[:, :, 0] - samples[:, :, 1]).astype(np.float32)  # (N, B)

    in_maps = []
    for c in range(NCORES):
        steps = c + NCORES * np.arange(NSTEP)
        W2c = (
            W2[steps]
            .reshape(NSTEP, NM, 128, H)
            .transpose(0, 2, 1, 3)
            .reshape(NSTEP, 128, NM * H)
            .astype(NPBF)
        )
        B1c = (
            b1[steps].reshape(NSTEP, NM, 128).transpose(2, 0, 1)
            .reshape(128, NSTEP * NM).astype(np.float32)
        )
        B2c = (
            b2[steps].reshape(NSTEP, NM, 128).transpose(2, 0, 1)
            .reshape(128, NSTEP * NM).astype(np.float32)
        )
        # W3D[p, ((j*NM + k)*NSTEP) + jj] = w3d[steps[j], k*128+p] if jj == j
        W3Dc = np.zeros((128, NSTEP, NM, NSTEP), np.float32)
        for j in range(NSTEP):
            W3Dc[:, j, :, j] = w3d[steps[j]].reshape(NM, 128).T
        W3Dc = W3Dc.reshape(128, NSTEP * NM * NSTEP).astype(NPBF)

        in_maps.append({
            "S": S,
            "W1": np.ascontiguousarray(W1m[steps]),
            "W2": W2c,
            "B1": B1c,
            "B2": B2c,
            "W3D": W3Dc,
            "B3D": b3d[steps].reshape(NSTEP, 1).astype(np.float32),
            "SIG": np.ascontiguousarray(sig[steps]),
        })
    return in_maps


def kernel(samples, W1, b1, W2, b2, W3, b3):
    global LAST_RESULT
    nc = _get_graph()
    in_maps = _prep_inputs(samples, W1, b1, W2, b2, W3, b3)
    res = run_bass_kernel_spmd(
        nc, in_maps, core_ids=list(range(NCORES)), trace=TRACE,
    )
    LAST_RESULT = res
    # out[c] rows are per-step logp; sum over steps and cores
    acc = np.zeros(B, np.float64)
    for c in range(NCORES):
        acc += np.asarray(res.results[c]["out"], np.float64).sum(axis=0)
    return acc.astype(np.float32).reshape(1, B)
